# revision 1
# baseline (speedup 1.0000x reference)
"""Causal multi-head attention (prefill) on 8 Trainium2 NeuronCores — v2.

Problem: x[2,2048,1024], Wq/Wk/Wv/Wo[1024,1024] (torch Linear [out,in]),
bo[1024]; y = MHA(x) with 16 heads of dim 64, causal softmax.

Sharding (data + tensor parallel): core c handles batch b=c//4 and head
group g=c%4 (4 heads = rows [256g, 256g+256) of Wq/Wk/Wv, cols of Wo).
Each core computes a partial y through its Wo column slice; the host sums
the 4 partials per batch and adds bo.

Changes vs the original baseline:
  - All inputs host-side pre-transposed and cast to bf16: xT[c,t],
    WqT/WkT/WvT[c,d], WoT[dpair,pr,dout]. No on-chip weight/x transposes.
  - All matmul operands bf16 (PE 1 cycle/row at any tile size; PSUM fp32).
  - Causal mask folded into the scores in PSUM: a constant [128,128]
    matmul seeds the diagonal block with -1e5 where q<k before the S
    matmul accumulates onto it, so exp emits exact zeros and no separate
    mask multiply (or its cross-engine hop) exists in the kt loop.
  - PV stays d-major (stationary [V|1] [128k,65], moving P^T) so each
    stationary load hides under a long moving stream; the flipped variant
    loses ~50us on HW to exposed ldweights despite a better cost-model PE
    time.
  - exp is the only ACT work; all PSUM->SBUF copies on DVE; psum pools
    split so next-chunk projections never wait on attention output tiles;
    output projection deferred to the end as PE filler for the ACT-paced
    late-chunk attention.
  - y written bf16; weights/constants DMA'd once outside the timing loop.
"""

import numpy as np
import ml_dtypes

import concourse.bass as bass
import concourse.mybir as mybir
import concourse.tile as tile
from concourse import bacc
from concourse.bass_utils import run_bass_kernel_spmd

P = 128
C = 1024
HD = 64
HPC = 4  # heads per core
NPAIR = 2  # head pairs per core
QB = 512  # q-block (PSUM bank width in fp32)
T_FULL = 2048
N_CORES = 8

f32 = mybir.dt.float32
bf16 = mybir.dt.bfloat16
AF = mybir.ActivationFunctionType
MUL = mybir.AluOpType.mult


def build_core_kernel(nc, tc, T, iters=1):
    TO = T // P  # t-tiles (16)
    CS = C // P  # c-subtiles (8)
    NQB = T // QB  # q-blocks (4)
    DS = HPC * HD  # 256, d-slice of this core

    xT_d = nc.dram_tensor("xT", [C, T], bf16, kind="ExternalInput").ap()
    wqT_d = nc.dram_tensor("wqT", [C, DS], bf16, kind="ExternalInput").ap()
    wkT_d = nc.dram_tensor("wkT", [C, DS], bf16, kind="ExternalInput").ap()
    wvT_d = nc.dram_tensor("wvT", [C, DS], bf16, kind="ExternalInput").ap()
    woT_d = nc.dram_tensor("woT", [P, NPAIR, C], bf16, kind="ExternalInput").ap()
    ident_d = nc.dram_tensor("ident", [P, P], bf16, kind="ExternalInput").ap()
    mneg_d = nc.dram_tensor("mneg", [P, P], bf16, kind="ExternalInput").ap()
    y_d = nc.dram_tensor("y", [T, C], bf16, kind="ExternalOutput").ap()

    persist_cm = tc.tile_pool(name="persist", bufs=1)
    persist = persist_cm.__enter__()

    ident = persist.tile([P, P], bf16, tag="ident")
    mneg = persist.tile([P, P], bf16, tag="mneg")
    wqT = persist.tile([P, CS, DS], bf16, tag="wqT")
    wkT = persist.tile([P, CS, DS], bf16, tag="wkT")
    wvT = persist.tile([P, CS, DS], bf16, tag="wvT")
    woT = persist.tile([P, NPAIR, C], bf16, tag="woT")
    qT = [persist.tile([P, T], bf16, tag=f"qT{p}", name=f"qT{p}") for p in range(NPAIR)]
    kT = [persist.tile([P, T], bf16, tag=f"kT{p}", name=f"kT{p}") for p in range(NPAIR)]
    vE = persist.tile([P, TO, HPC, HD + 1], bf16, tag="vE")
    outT = [
        persist.tile([P, T], bf16, tag=f"outT{p}", name=f"outT{p}")
        for p in range(NPAIR)
    ]

    # loop-invariant loads (outside the timing loop)
    nc.sync.dma_start(ident[:], ident_d)
    nc.sync.dma_start(mneg[:], mneg_d)
    for w_src, w_dst in ((wqT_d, wqT), (wkT_d, wkT), (wvT_d, wvT)):
        nc.sync.dma_start(w_dst[:], w_src.rearrange("(cs p) d -> p cs d", p=P))
    nc.sync.dma_start(woT[:], woT_d)
    # ones column of [V|1]
    nc.gpsimd.memset(vE[:, :, :, HD : HD + 1], 1.0)

    import contextlib

    loop_cm = (
        tc.For_i(0, iters, 1, hint_engines=(mybir.EngineType.PE,))
        if iters > 1
        else contextlib.nullcontext()
    )
    with loop_cm:
        _body(nc, tc, T, locals())

    persist_cm.__exit__(None, None, None)


def _body(nc, tc, T, env):
    TO, CS, NQB, DS = env["TO"], env["CS"], env["NQB"], env["DS"]
    xT_d, y_d = env["xT_d"], env["y_d"]
    ident, mneg = env["ident"], env["mneg"]
    wqT, wkT, wvT, woT = env["wqT"], env["wkT"], env["wvT"], env["woT"]
    qT, kT, vE, outT = env["qT"], env["kT"], env["vE"], env["outT"]

    xT_r = xT_d.rearrange("(cs p) t -> p cs t", p=P)

    with (
        tc.tile_pool(name="xt_pool", bufs=2) as xt_pool,
        tc.tile_pool(name="psum_m", bufs=2, space="PSUM") as psum_m,
        tc.tile_pool(name="psum_s", bufs=2, space="PSUM") as psum_s,
        tc.tile_pool(name="psum_o", bufs=2, space="PSUM") as psum_o,
        tc.tile_pool(name="pt_pool", bufs=6) as pt_pool,
        tc.tile_pool(name="sb_norm", bufs=2) as sb_norm,
        tc.tile_pool(name="sb_y", bufs=8) as sb_y,
    ):
        for jc in range(NQB):
            # ---- stream x chunk jc (c-major, pre-transposed on host)
            xTc = xt_pool.tile([P, CS, QB], bf16, tag="xTc")
            nc.sync.dma_start(xTc[:], xT_r[:, :, jc * QB : (jc + 1) * QB])

            # ---- Q^T / K^T for this chunk (d-major, pair-stacked)
            for pr in range(NPAIR):
                for wT, dstT in ((wqT, qT[pr]), (wkT, kT[pr])):
                    pp = psum_m.tile([P, QB], f32, tag="m", name="ppqk")
                    for cs in range(CS):
                        nc.tensor.matmul(
                            pp[:],
                            wT[:, cs, pr * P : (pr + 1) * P],
                            xTc[:, cs, :],
                            start=(cs == 0),
                            stop=(cs == CS - 1),
                        )
                    nc.vector.tensor_copy(dstT[:, jc * QB : (jc + 1) * QB], pp[:])
            # ---- V (t-major) for this chunk's 4 t-tiles
            for ol in range(QB // P):
                tt = jc * (QB // P) + ol
                vp = psum_m.tile([P, DS], f32, tag="m", name="ppv")
                for cs in range(CS):
                    nc.tensor.matmul(
                        vp[:],
                        xTc[:, cs, ol * P : (ol + 1) * P],
                        wvT[:, cs, :],
                        start=(cs == 0),
                        stop=(cs == CS - 1),
                    )
                nc.vector.tensor_copy(
                    vE[:, tt, :, 0:HD],
                    vp[:].rearrange("p (h d) -> p h d", h=HPC),
                )

            # ---- attention for q-block qb = jc, both pairs
            qb = jc
            nkt = 4 * qb + 4
            for pr in range(NPAIR):
                oext = [
                    psum_o.tile([HD + 1, QB], f32, tag="oe", name=f"oe{_i}")
                    for _i in range(2)
                ]
                for kt in range(nkt):
                    s = kt - 4 * qb
                    qoff = max(s, 0) * P
                    st_ = psum_s.tile([P, 2, QB], f32, tag="s", name="st_")
                    for hi in range(2):
                        hsel = slice(hi * HD, (hi + 1) * HD)
                        if s >= 0:
                            # causal mask folded into PSUM: seed the diagonal
                            # [128,128] block with -1e5 where q < k BEFORE the
                            # S matmul accumulates onto it; exp then emits
                            # exact zeros there. Constant operands, so this is
                            # never on the qT/kT critical path.
                            nc.tensor.matmul(
                                st_[:, hi, qoff : qoff + P],
                                ident,
                                mneg,
                                start=True,
                                stop=False,
                            )
                        nc.tensor.matmul(
                            st_[:, hi, qoff:QB],
                            kT[pr][hsel, kt * P : (kt + 1) * P],
                            qT[pr][hsel, qb * QB + qoff : (qb + 1) * QB],
                            start=(s < 0),
                            stop=True,
                            tile_position=(hi * HD, 0),
                        )
                    pt = pt_pool.tile([P, 2, QB], bf16, tag="pT")
                    nc.scalar.activation(
                        pt[:, :, qoff:QB], st_[:, :, qoff:QB], AF.Exp, scale=0.125
                    )
                    # d-major PV: stationary [V|1] [128,65] streams 512-qoff
                    # columns of P^T, so ldweights hides under the stream
                    for hi in range(2):
                        h = pr * 2 + hi
                        nc.tensor.matmul(
                            oext[hi][:, qoff:QB],
                            vE[:, kt, h, :],
                            pt[:, hi, qoff:QB],
                            start=(kt == 0),
                            stop=(kt == nkt - 1),
                        )
                # ---- normalize: recip of denominators (row HD), broadcast
                # across the head's 64 partitions, multiply -> d-major outT
                for hi in range(2):
                    rc = sb_norm.tile([1, QB], f32, tag=f"rc{hi}", name=f"rc{hi}")
                    nc.vector.reciprocal(rc[:], oext[hi][HD : HD + 1, :])
                    rs = sb_norm.tile([HD, QB], f32, tag=f"rs{hi}", name=f"rs{hi}")
                    nc.gpsimd.partition_broadcast(rs[:], rc[:], channels=HD)
                    nc.vector.tensor_tensor(
                        outT[pr][hi * HD : (hi + 1) * HD, qb * QB : (qb + 1) * QB],
                        oext[hi][0:HD, :],
                        rs[:],
                        MUL,
                    )

        # ---- output projection, emitted after all chunks: the scheduler uses
        # these as PE filler during the ACT-paced late-chunk attention.
        for tt in range(TO):
            yv = sb_y.tile([P, C], bf16, tag="yv")
            for doc in range(C // QB):
                # psum_m is free of projection traffic after the last chunk,
                # and y must stay out of the S-tile ring to not stall ACT
                yp = psum_m.tile([P, QB], f32, tag="m", name="yp")
                for pr in range(NPAIR):
                    nc.tensor.matmul(
                        yp[:],
                        outT[pr][:, tt * P : (tt + 1) * P],
                        woT[:, pr, doc * QB : (doc + 1) * QB],
                        start=(pr == 0),
                        stop=(pr == NPAIR - 1),
                    )
                nc.vector.tensor_copy(yv[:, doc * QB : (doc + 1) * QB], yp[:])
            nc.sync.dma_start(y_d[tt * P : (tt + 1) * P, :], yv[:])


def build_nc(T=T_FULL, iters=1):
    nc = bacc.Bacc("TRN2", target_bir_lowering=False, debug=False, num_devices=N_CORES)
    with tile.TileContext(nc) as tc:
        build_core_kernel(nc, tc, T, iters=iters)
    nc.compile()
    return nc


def make_consts():
    ident = np.eye(P, dtype=ml_dtypes.bfloat16)
    k = np.arange(P)
    # mneg[k,q] = -1e5 where q < k (causal mask additive seed), else 0
    mneg = np.where(k[None, :] < k[:, None], -1e5, 0.0).astype(ml_dtypes.bfloat16)
    return ident, mneg


def make_in_maps(x, Wq, Wk, Wv, Wo):
    """Per-core input dicts. Core c: batch c//4, head group c%4."""
    ident, mneg = make_consts()
    bf = ml_dtypes.bfloat16
    in_maps = []
    for c in range(N_CORES):
        b, g = divmod(c, 4)
        ds = slice(g * 256, (g + 1) * 256)
        woT = np.ascontiguousarray(
            Wo[:, ds].T.reshape(NPAIR, P, C).transpose(1, 0, 2)
        ).astype(bf)
        in_maps.append(
            {
                "xT": np.ascontiguousarray(x[b].T).astype(bf),
                "wqT": np.ascontiguousarray(Wq[ds, :].T).astype(bf),
                "wkT": np.ascontiguousarray(Wk[ds, :].T).astype(bf),
                "wvT": np.ascontiguousarray(Wv[ds, :].T).astype(bf),
                "woT": woT,
                "ident": ident,
                "mneg": mneg,
            }
        )
    return in_maps


def gather(results, bo):
    """Sum partial outputs per batch, add bias."""
    B = N_CORES // 4
    y = np.zeros((B, T_FULL, C), dtype=np.float32)
    for c in range(N_CORES):
        y[c // 4] += results[c]["y"].astype(np.float32)
    y += bo.astype(np.float32)
    return y.astype(np.float32)


_NC_CACHE = {}


def get_nc():
    if "nc" not in _NC_CACHE:
        _NC_CACHE["nc"] = build_nc()
    return _NC_CACHE["nc"]


def kernel(x, Wq, Wk, Wv, Wo, bo):
    x = np.asarray(x, dtype=np.float32)
    Wq = np.asarray(Wq, dtype=np.float32)
    Wk = np.asarray(Wk, dtype=np.float32)
    Wv = np.asarray(Wv, dtype=np.float32)
    Wo = np.asarray(Wo, dtype=np.float32)
    bo = np.asarray(bo, dtype=np.float32)
    nc = get_nc()
    in_maps = make_in_maps(x, Wq, Wk, Wv, Wo)
    res = run_bass_kernel_spmd(nc, in_maps, core_ids=list(range(N_CORES)))
    return gather(res.results, bo)

